# revision 44
# baseline (speedup 1.0000x reference)
"""Trainium2 Bass kernel for nn_CoreAttention (S=2048, B=1, H=16, D=128).

Sharding: 16 heads across 8 NeuronCores (2 heads/core, tensor parallel).

v3 design: natural-output PV with a fused softmax-sums column (129th
moving column of ones), causal scores stream with exp on ACT, and
aggressive cross-head software pipelining:
  - input DMAs chunked small-first and priority-ordered (one dma_start
    lands on ONE dma engine at ~22GB/s, so lead chunks are 128 cols)
  - head-1 q/k projections interleaved into head-0's scores stream
  - psum->sbuf drains split between DVE and ACT to avoid serializing
    against the projection matmuls
  - per-head output staging tile so out-DMA issue order never
    backpressures the band epilogues
"""

import sys
from contextlib import ExitStack

import numpy as np

for _p in ("/opt/trn_rl_repo",):
    if _p not in sys.path:
        sys.path.insert(0, _p)

import ml_dtypes
import concourse.bass as bass
import concourse.tile as tile
from concourse import bacc, mybir
from concourse.bass_utils import run_bass_kernel_spmd

S, B, H, D = 2048, 1, 16, 128
HPC = 2  # heads per core
NCORES = 8
NB = S // 128  # 16 seq blocks of 128
NF = float(np.sqrt(2048.0 / 16.0))  # NORM_FACTOR
NEG = -10000.0
TILE = 1024  # psum scores tile columns (2 banks)

F32 = mybir.dt.float32
F32R = mybir.dt.float32r
BF16 = mybir.dt.bfloat16
AF = mybir.ActivationFunctionType

# causal column stream: pass j emits scoresT columns for q in [128j, S)
OFFS = [0]
for j in range(NB):
    OFFS.append(OFFS[-1] + (S - 128 * j))
W = OFFS[NB]  # 17408 total causal columns per head


def build_program() -> bass.Bass:
    nc = bacc.Bacc(
        "TRN2", target_bir_lowering=False, debug=False, num_devices=NCORES
    )

    qt_d = nc.dram_tensor("qt", [HPC, D, S], BF16, kind="ExternalInput")
    kt_d = nc.dram_tensor("kt", [HPC, D, S], BF16, kind="ExternalInput")
    vt_d = nc.dram_tensor("vt", [HPC, D, S], BF16, kind="ExternalInput")
    wqk_d = nc.dram_tensor("wqk", [HPC, D, D], BF16, kind="ExternalInput")
    wv_d = nc.dram_tensor("wv", [HPC, D, D], BF16, kind="ExternalInput")
    mask01_d = nc.dram_tensor("mask01", [D, D], BF16, kind="ExternalInput")
    out_d = nc.dram_tensor("out", [HPC, S, D], BF16, kind="ExternalOutput")

    def make_tiles(lead_small, tail_cuts):
        """Column tiling of the causal stream.  Small lead tiles let the
        stream start while the bulk of q is still in flight; tail cuts
        at the last bands' diagonals release PV(13..15) progressively."""
        widths = ([512, 512, 512] if lead_small else [])
        pos = sum(widths)
        while pos < W:
            w = min(TILE, W - pos)
            widths.append(w)
            pos += w
        tiles = []
        pos = 0
        for w in widths:
            if tail_cuts:
                for cut in (OFFS[13], OFFS[14], OFFS[15]):
                    if pos < cut < pos + w:
                        tiles.append((pos, cut - pos))
                        w -= cut - pos
                        pos = cut
            tiles.append((pos, w))
            pos += w
        band_tile = []
        for i in range(NB):
            for t, (t0, w) in enumerate(tiles):
                if t0 <= OFFS[i] < t0 + w:
                    band_tile.append(t)
                    break
        return tiles, band_tile

    with tile.TileContext(nc) as tc, ExitStack() as ctx:
        cpool = ctx.enter_context(tc.tile_pool(name="const", bufs=1))
        sb = ctx.enter_context(tc.tile_pool(name="sb", bufs=1))
        ps = ctx.enter_context(tc.tile_pool(name="ps", bufs=1, space="PSUM"))

        # ---- input DMAs: h0's critical path leads both HWDGE queues ---
        # SP: wqk0 + q0 lead/bulk, v0, then all of h1.  ACT: k0 + consts
        # (ACT queue starts in parallel with SP's, and ACT is idle until
        # the first exp).
        wqk_t, qkraw, vtb, wvb = {}, {}, {}, {}
        for h in range(HPC):
            wqk_t[h] = sb.tile([D, D], BF16, tag="wqk", bufs=2, name=f"wqk_{h}")
            wvb[h] = sb.tile([D, D], BF16, tag="wvb", bufs=2, name=f"wvb_{h}")
            qkraw[h] = sb.tile([D, 2 * S], BF16, tag="qkraw", bufs=2,
                               name=f"qkraw_{h}")
            vtb[h] = sb.tile([D, S], BF16, tag="vtb", bufs=2, name=f"vtb_{h}")
        mask01 = cpool.tile([D, D], BF16)
        nc.scalar.dma_start(mask01[:], mask01_d[:])
        nc.scalar.dma_start(qkraw[0][:, S : S + 256], kt_d[0][:, 0:256])
        nc.scalar.dma_start(qkraw[0][:, S + 256 : S + 1024],
                            kt_d[0][:, 256:1024])
        nc.scalar.dma_start(qkraw[0][:, S + 1024 : 2 * S],
                            kt_d[0][:, 1024:2048])
        nc.sync.dma_start(wqk_t[0][:], wqk_d[0])
        nc.sync.dma_start(qkraw[0][:, 0:256], qt_d[0][:, 0:256])
        nc.sync.dma_start(qkraw[0][:, 256:1024], qt_d[0][:, 256:1024])
        nc.sync.dma_start(qkraw[0][:, 1024:2048], qt_d[0][:, 1024:2048])
        nc.sync.dma_start(wvb[0][:], wv_d[0])
        nc.sync.dma_start(vtb[0][:], vt_d[0])
        nc.sync.dma_start(wqk_t[1][:], wqk_d[1])
        nc.sync.dma_start(wvb[1][:], wv_d[1])
        nc.sync.dma_start(qkraw[1][:, 0:2048], qt_d[1])
        nc.sync.dma_start(qkraw[1][:, S : 2 * S], kt_d[1])
        nc.sync.dma_start(vtb[1][:], vt_d[1])

        # ---- working tiles; gpsimd only seeds the vaug ones column ----
        vaug, qkmt, exp_all, osb = {}, {}, {}, {}
        for h in range(HPC):
            vaug[h] = sb.tile([D, NB, 132], BF16, tag="vaug", bufs=2,
                              name=f"vaug_{h}")
            qkmt[h] = sb.tile([D, 2 * S], BF16, tag="qkmt", bufs=2,
                              name=f"qkmt_{h}")
            exp_all[h] = sb.tile([D, W], BF16, tag="expall", bufs=2,
                                 name=f"expall_{h}")
            osb[h] = sb.tile([D, NB, 128], BF16, tag="osb", bufs=2,
                             name=f"osb_{h}")
            nc.gpsimd.memset(vaug[h][:, :, 128:129], 1.0)

        # ---- PE p-state warmup: garbage matmuls fill the DMA-wait ----
        # window so the PE is at full clock when real data lands
        junk = sb.tile([D, 512], BF16, tag="junk", bufs=1, name="junk")
        nc.gpsimd.memset(junk[:], 1.0)
        for wi in range(8):
            warm_ps = ps.tile([D, TILE], F32, tag="big", bufs=3,
                              name=f"warm_{wi}")
            nc.tensor.matmul(warm_ps[:, 0:512], junk[:, 0:128], junk[:],
                             start=True, stop=True)

        def proj_chunk(h, part, c0, w, copy_eng):
            """one bf16 projection chunk psum tile + drain copy."""
            base = part * S
            bigt = ps.tile([D, TILE], F32, tag="big", bufs=3,
                           name=f"projps_{h}_{part}_{c0}")
            for c in range(0, w, 512):
                cw = min(512, w - c)
                nc.tensor.matmul(
                    bigt[:, c : c + cw],
                    wqk_t[h][:],
                    qkraw[h][:, base + c0 + c : base + c0 + c + cw],
                    start=True,
                    stop=True,
                )
            if copy_eng == "act":
                nc.scalar.activation(
                    qkmt[h][:, base + c0 : base + c0 + w], bigt[:, 0:w],
                    AF.Copy,
                )
            else:
                nc.vector.tensor_copy(
                    qkmt[h][:, base + c0 : base + c0 + w], bigt[:, 0:w]
                )

        def emit_qk_proj_h0():
            """head-0 projections matched to DMA chunk arrival: q/k leads
            (k-lead copy on pre-stream-idle ACT), then bulk.  h0 vtb
            casts interleave on DVE."""
            proj_chunk(0, 0, 0, 256, "dve")
            proj_chunk(0, 1, 0, 256, "act")
            proj_chunk(0, 0, 256, 768, "dve")
            proj_chunk(0, 0, 1024, 1024, "dve")
            proj_chunk(0, 1, 256, 768, "dve")
            proj_chunk(0, 1, 1024, 1024, "dve")

        def emit_vproj(h, j0, nj):
            """v chunks j0..j0+nj-1 -> vaug natural layout."""
            w = nj * 128
            bigt = ps.tile([D, TILE], F32, tag="big", bufs=3,
                           name=f"vps_{h}_{j0}")
            for c in range(0, w, 128):
                nc.tensor.matmul(
                    bigt[:, c : c + 128],
                    vtb[h][:, j0 * 128 + c : j0 * 128 + c + 128],
                    wvb[h][:],
                    start=True,
                    stop=True,
                )
            nc.vector.tensor_copy(
                vaug[h][:, j0 : j0 + nj, 0:128],
                bigt[:, 0:w].rearrange("p (j c) -> p j c", c=128),
            )

        ctx_box = [None]

        def emit_pv(h, i, last_head):
            if i % 2 == 0:
                ctx_box[0] = ps.tile([D, 2, 132], F32, tag="ctx", bufs=2,
                                     name=f"ctx_{h}_{i}")
            ctxp = ctx_box[0]
            sl = i % 2
            for j in range(i + 1):
                p = OFFS[j] + 128 * (i - j)
                nc.tensor.matmul(
                    ctxp[:, sl, 0:129],
                    exp_all[h][:, p : p + 128],
                    vaug[h][:, j, 0:129],
                    start=(j == 0),
                    stop=(j == i),
                )
            recip = sb.tile([D, 1], F32, tag="recip", bufs=4,
                            name=f"recip_{h}_{i}")
            nc.vector.reciprocal(recip[:], ctxp[:, sl, 128:129])
            nc.vector.tensor_scalar_mul(
                osb[h][:, i, :], ctxp[:, sl, 0:128], recip[:]
            )
            # milestone out-DMAs; split the tail bands on the last head so
            # the final DMA is small
            def dma_bands(b0, b1):
                nc.sync.dma_start(
                    out_d[h, b0 * 128 : b1 * 128, :].rearrange(
                        "(b s) e -> s b e", b=b1 - b0
                    ),
                    osb[h][:, b0:b1, :],
                )
            if not last_head:
                if i in (3, 7, 11, 15):
                    dma_bands(i - 3, i + 1)
            else:
                # fire per band from 8 on so the final transfer is tiny;
                # late ones go out the otherwise-idle ACT queue
                if i in (3, 7):
                    dma_bands(i - 3, i + 1)
                elif 8 <= i <= 9:
                    dma_bands(i, i + 1)
                elif 10 <= i <= 14:
                    nc.scalar.dma_start(
                        out_d[h, i * 128 : (i + 1) * 128, :], osb[h][:, i, :]
                    )
                elif i == 15:
                    nc.scalar.dma_start(out_d[h, 15 * 128 : 15 * 128 + 128, 0:64],
                                        osb[h][:, 15, 0:64])
                    nc.scalar.dma_start(out_d[h, 15 * 128 : 15 * 128 + 128, 64:128],
                                        osb[h][:, 15, 64:128])

        def emit_score_tile(h, tiles, t):
            t0, w = tiles[t]
            bigt = ps.tile([D, TILE], F32, tag="big", bufs=3,
                           name=f"scps_{h}_{t}")
            pos = t0
            while pos < t0 + w:
                j = 0
                while OFFS[j + 1] <= pos:
                    j += 1
                bank_end = t0 + ((pos - t0) // 512 + 1) * 512
                end = min(OFFS[j + 1], t0 + w, bank_end)
                qcol = 128 * j + (pos - OFFS[j])
                nc.tensor.matmul(
                    bigt[:, pos - t0 : end - t0],
                    qkmt[h][:, S + 128 * j : S + 128 * (j + 1)],
                    qkmt[h][:, qcol : qcol + (end - pos)],
                    start=True,
                    stop=True,
                )
                pos = end
            nc.scalar.activation(
                exp_all[h][:, t0 : t0 + w], bigt[:, 0:w], AF.Exp,
                scale=1.0 / NF,
            )
            # causal mask on diagonal blocks: zero the upper triangle of
            # exp via a DVE 0/1 multiply (keeps the PE free of mask work)
            for j in range(NB):
                ds = OFFS[j]
                if t0 <= ds < t0 + w:
                    nc.vector.scalar_tensor_tensor(
                        exp_all[h][:, ds : ds + 128],
                        exp_all[h][:, ds : ds + 128],
                        1.0,
                        mask01[:],
                        op0=mybir.AluOpType.mult,
                        op1=mybir.AluOpType.mult,
                    )

        def emit_stream(h, tiles, band_tile, interleave, pv_start_t,
                        first_t=0):
            """scores/exp/PV stream for head h.  `interleave` maps tile
            index -> list of callables to emit after that tile's exp.
            Tiles < first_t were already emitted elsewhere."""
            issued = 0  # bands 0..issued-1 already emitted
            for t in range(first_t, len(tiles)):
                emit_score_tile(h, tiles, t)
                for fn in interleave.get(t, ()):
                    fn()
                if t >= pv_start_t:
                    while issued < NB and band_tile[issued] <= t - 1:
                        emit_pv(h, issued, h == HPC - 1)
                        issued += 1
            while issued < NB:
                emit_pv(h, issued, h == HPC - 1)
                issued += 1

        # ---- head 0: proj pre-stream, h1-proj interleaved late --------
        emit_qk_proj_h0()

        def h1_proj_piece(part, half):
            def fn():
                base = part * S
                bigt = ps.tile([D, TILE], F32, tag="big", bufs=3,
                               name=f"projps1_{part}_{half}")
                for c in range(2):
                    c0 = half * 1024 + c * 512
                    nc.tensor.matmul(
                        bigt[:, c * 512 : c * 512 + 512],
                        wqk_t[1][:],
                        qkraw[1][:, base + c0 : base + c0 + 512],
                        start=True,
                        stop=True,
                    )
                nc.vector.tensor_copy(
                    qkmt[1][:, base + half * 1024 : base + half * 1024 + 1024],
                    bigt[:, 0:1024],
                )
            return fn

        tiles0, band_tile0 = make_tiles(lead_small=True, tail_cuts=False)
        tiles1, band_tile1 = make_tiles(lead_small=False, tail_cuts=False)
        n0 = len(tiles0)
        inter0 = {}

        def add_inter(t, fn):
            inter0.setdefault(t, []).append(fn)

        add_inter(3, lambda: emit_vproj(0, 0, 8))
        add_inter(4, lambda: emit_vproj(0, 8, 8))
        add_inter(6, h1_proj_piece(0, 0))
        add_inter(7, h1_proj_piece(0, 1))
        add_inter(8, h1_proj_piece(1, 0))
        add_inter(9, h1_proj_piece(1, 1))
        add_inter(11, lambda: emit_vproj(1, 0, 8))
        add_inter(12, lambda: emit_vproj(1, 8, 8))
        # feed h1's first score tiles into h0's tail so ACT never idles
        # across the head transition
        for k in range(4):
            add_inter(n0 - 4 + k,
                      (lambda kk: lambda: emit_score_tile(1, tiles1, kk))(k))
        emit_stream(0, tiles0, band_tile0, inter0, pv_start_t=4)

        # ---- head 1 (proj, vproj, tiles 0-4 already emitted above) ----
        emit_score_tile(1, tiles1, 4)
        emit_stream(1, tiles1, band_tile1, {}, pv_start_t=5, first_t=5)

    nc.compile()
    return nc


_NC_CACHE = None


def _get_program():
    global _NC_CACHE
    if _NC_CACHE is None:
        _NC_CACHE = build_program()
    return _NC_CACHE


def make_in_maps(query_layer, key_layer, value_layer, svd_qk, svd_v):
    bf = ml_dtypes.bfloat16
    qt = np.ascontiguousarray(query_layer[:, 0].transpose(1, 2, 0)).astype(bf)
    kt = np.ascontiguousarray(key_layer[:, 0].transpose(1, 2, 0)).astype(bf)
    vt = np.ascontiguousarray(value_layer[:, 0].transpose(1, 2, 0)).astype(bf)
    svd_qk = np.ascontiguousarray(svd_qk).astype(bf)
    svd_v = np.ascontiguousarray(svd_v).astype(bf)

    r = np.arange(D)
    mask01 = np.where(r[:, None] > r[None, :], 0.0, 1.0).astype(ml_dtypes.bfloat16)

    in_maps = []
    for c in range(NCORES):
        hs = slice(c * HPC, (c + 1) * HPC)
        in_maps.append(
            {
                "qt": qt[hs],
                "kt": kt[hs],
                "vt": vt[hs],
                "wqk": svd_qk[hs],
                "wv": svd_v[hs],
                "mask01": mask01,
            }
        )
    return in_maps


def assemble_output(results):
    out = np.empty((S, B, H * D), dtype=np.float32)
    for c in range(NCORES):
        o = np.asarray(results[c]["out"], dtype=np.float32)  # [HPC, S, D]
        for hl in range(HPC):
            h = c * HPC + hl
            out[:, 0, h * D : (h + 1) * D] = o[hl]
    return out


def kernel(query_layer, key_layer, value_layer, attention_mask, svd_qk, svd_v):
    nc = _get_program()
    in_maps = make_in_maps(query_layer, key_layer, value_layer, svd_qk, svd_v)
    res = run_bass_kernel_spmd(nc, in_maps, list(range(NCORES))).results
    return assemble_output(res)


# revision 46
# speedup vs baseline: 1.0326x; 1.0326x over previous
"""Trainium2 Bass kernel for nn_CoreAttention (S=2048, B=1, H=16, D=128).

Sharding: 16 heads across 8 NeuronCores (2 heads/core, tensor parallel).

Design: flash-style causal stream per head with a natural-output PV.

  qk_proj: psum = Wqk^T @ [Q^T | K^T]       (bf16 PE, host pre-casts all
                                             inputs to bf16)
  v:       v_nat[s,e] = V^T-chunk^T @ Wv    stored as vaug[k, j, 0:129]
                                             with column 128 = 1.0
  scoresT[k,q] = kmt_j^T @ qmt chunks       (causal column stream in
                                             1024-col psum tiles x3;
                                             diag mask via PE accumulate
                                             of a -1e4 tile)
  expT   = exp(scoresT / NF)                (ACT, scale folded in, bf16)
  PV band i: ctx[q, 0:129] = sum_j expT_block(i,j)^T @ vaug_j
             column 128 = the softmax sums (ones column of vaug)
  out    = ctx[:, 0:128] * (1/ctx[:, 128])  (DVE recip + per-partition
                                             scalar multiply, bf16 out,
                                             host upcasts)

Schedule highlights (all tuned against neuron-profile traces):
  - input DMAs split across the SP and ACT HWDGE queues, lead chunks
    first, so the stream starts ~8us in; PE p-state warmup matmuls fill
    the DMA-wait window
  - head-1's projections, v-projection and first score tiles are
    interleaved into head-0's stream so ACT (the pacing engine, ~39us
    of exp) never idles across the head transition
  - PV bands pair up in [128, 2, 132] psum tiles; per-head output
    staging so out-DMA issue never backpressures the epilogue
"""

import sys
from contextlib import ExitStack

import numpy as np

for _p in ("/opt/trn_rl_repo",):
    if _p not in sys.path:
        sys.path.insert(0, _p)

import ml_dtypes
import concourse.bass as bass
import concourse.tile as tile
from concourse import bacc, mybir
from concourse.bass_utils import run_bass_kernel_spmd

S, B, H, D = 2048, 1, 16, 128
HPC = 2  # heads per core
NCORES = 8
NB = S // 128  # 16 seq blocks of 128
NF = float(np.sqrt(2048.0 / 16.0))  # NORM_FACTOR
NEG = -10000.0
TILE = 1024  # psum scores tile columns (2 banks)

F32 = mybir.dt.float32
F32R = mybir.dt.float32r
BF16 = mybir.dt.bfloat16
AF = mybir.ActivationFunctionType

# causal column stream: pass j emits scoresT columns for q in [128j, S)
OFFS = [0]
for j in range(NB):
    OFFS.append(OFFS[-1] + (S - 128 * j))
W = OFFS[NB]  # 17408 total causal columns per head


def build_program() -> bass.Bass:
    nc = bacc.Bacc(
        "TRN2", target_bir_lowering=False, debug=False, num_devices=NCORES
    )

    qt_d = nc.dram_tensor("qt", [HPC, D, S], BF16, kind="ExternalInput")
    kt_d = nc.dram_tensor("kt", [HPC, D, S], BF16, kind="ExternalInput")
    vt_d = nc.dram_tensor("vt", [HPC, D, S], BF16, kind="ExternalInput")
    wqk_d = nc.dram_tensor("wqk", [HPC, D, D], BF16, kind="ExternalInput")
    wv_d = nc.dram_tensor("wv", [HPC, D, D], BF16, kind="ExternalInput")
    identb_d = nc.dram_tensor("identb", [D, D], BF16, kind="ExternalInput")
    maskb_d = nc.dram_tensor("maskb", [D, D], BF16, kind="ExternalInput")
    out_d = nc.dram_tensor("out", [HPC, S, D], BF16, kind="ExternalOutput")

    def make_tiles(lead_small, tail_cuts):
        """Column tiling of the causal stream.  Small lead tiles let the
        stream start while the bulk of q is still in flight; tail cuts
        at the last bands' diagonals release PV(13..15) progressively."""
        widths = ([512, 512, 512] if lead_small else [])
        pos = sum(widths)
        while pos < W:
            w = min(TILE, W - pos)
            widths.append(w)
            pos += w
        tiles = []
        pos = 0
        for w in widths:
            if tail_cuts:
                for cut in (OFFS[13], OFFS[14], OFFS[15]):
                    if pos < cut < pos + w:
                        tiles.append((pos, cut - pos))
                        w -= cut - pos
                        pos = cut
            tiles.append((pos, w))
            pos += w
        band_tile = []
        for i in range(NB):
            for t, (t0, w) in enumerate(tiles):
                if t0 <= OFFS[i] < t0 + w:
                    band_tile.append(t)
                    break
        return tiles, band_tile

    with tile.TileContext(nc) as tc, ExitStack() as ctx:
        cpool = ctx.enter_context(tc.tile_pool(name="const", bufs=1))
        sb = ctx.enter_context(tc.tile_pool(name="sb", bufs=1))
        ps = ctx.enter_context(tc.tile_pool(name="ps", bufs=1, space="PSUM"))

        # ---- input DMAs: h0's critical path leads both HWDGE queues ---
        # SP: wqk0 + q0 lead/bulk, v0, then all of h1.  ACT: k0 + consts
        # (ACT queue starts in parallel with SP's, and ACT is idle until
        # the first exp).
        wqk_t, qkraw, vtb, wvb = {}, {}, {}, {}
        for h in range(HPC):
            wqk_t[h] = sb.tile([D, D], BF16, tag="wqk", bufs=2, name=f"wqk_{h}")
            wvb[h] = sb.tile([D, D], BF16, tag="wvb", bufs=2, name=f"wvb_{h}")
            qkraw[h] = sb.tile([D, 2 * S], BF16, tag="qkraw", bufs=2,
                               name=f"qkraw_{h}")
            vtb[h] = sb.tile([D, S], BF16, tag="vtb", bufs=2, name=f"vtb_{h}")
        identb = cpool.tile([D, D], BF16)
        nc.scalar.dma_start(identb[:], identb_d[:])
        maskb = cpool.tile([D, D], BF16)
        nc.scalar.dma_start(maskb[:], maskb_d[:])
        nc.scalar.dma_start(qkraw[0][:, S : S + 256], kt_d[0][:, 0:256])
        nc.scalar.dma_start(qkraw[0][:, S + 256 : S + 1024],
                            kt_d[0][:, 256:1024])
        nc.scalar.dma_start(qkraw[0][:, S + 1024 : 2 * S],
                            kt_d[0][:, 1024:2048])
        nc.sync.dma_start(wqk_t[0][:], wqk_d[0])
        nc.sync.dma_start(qkraw[0][:, 0:256], qt_d[0][:, 0:256])
        nc.sync.dma_start(qkraw[0][:, 256:1024], qt_d[0][:, 256:1024])
        nc.sync.dma_start(qkraw[0][:, 1024:2048], qt_d[0][:, 1024:2048])
        nc.sync.dma_start(wvb[0][:], wv_d[0])
        nc.sync.dma_start(vtb[0][:], vt_d[0])
        nc.sync.dma_start(wqk_t[1][:], wqk_d[1])
        nc.sync.dma_start(wvb[1][:], wv_d[1])
        nc.sync.dma_start(qkraw[1][:, 0:2048], qt_d[1])
        nc.sync.dma_start(qkraw[1][:, S : 2 * S], kt_d[1])
        nc.sync.dma_start(vtb[1][:], vt_d[1])

        # ---- working tiles; gpsimd only seeds the vaug ones column ----
        vaug, qkmt, exp_all, osb = {}, {}, {}, {}
        for h in range(HPC):
            vaug[h] = sb.tile([D, NB, 132], BF16, tag="vaug", bufs=2,
                              name=f"vaug_{h}")
            qkmt[h] = sb.tile([D, 2 * S], BF16, tag="qkmt", bufs=2,
                              name=f"qkmt_{h}")
            exp_all[h] = sb.tile([D, W], BF16, tag="expall", bufs=2,
                                 name=f"expall_{h}")
            osb[h] = sb.tile([D, NB, 128], BF16, tag="osb", bufs=2,
                             name=f"osb_{h}")
            nc.gpsimd.memset(vaug[h][:, :, 128:129], 1.0)

        # ---- PE p-state warmup: garbage matmuls fill the DMA-wait ----
        # window so the PE is at full clock when real data lands
        junk = sb.tile([D, 512], BF16, tag="junk", bufs=1, name="junk")
        nc.gpsimd.memset(junk[:], 1.0)
        for wi in range(8):
            warm_ps = ps.tile([D, TILE], F32, tag="big", bufs=3,
                              name=f"warm_{wi}")
            nc.tensor.matmul(warm_ps[:, 0:512], junk[:, 0:128], junk[:],
                             start=True, stop=True)

        def proj_chunk(h, part, c0, w, copy_eng):
            """one bf16 projection chunk psum tile + drain copy."""
            base = part * S
            bigt = ps.tile([D, TILE], F32, tag="big", bufs=3,
                           name=f"projps_{h}_{part}_{c0}")
            for c in range(0, w, 512):
                cw = min(512, w - c)
                nc.tensor.matmul(
                    bigt[:, c : c + cw],
                    wqk_t[h][:],
                    qkraw[h][:, base + c0 + c : base + c0 + c + cw],
                    start=True,
                    stop=True,
                )
            if copy_eng == "act":
                nc.scalar.activation(
                    qkmt[h][:, base + c0 : base + c0 + w], bigt[:, 0:w],
                    AF.Copy,
                )
            else:
                nc.vector.tensor_copy(
                    qkmt[h][:, base + c0 : base + c0 + w], bigt[:, 0:w]
                )

        def emit_qk_proj_h0():
            """head-0 projections matched to DMA chunk arrival: q/k leads
            (k-lead copy on pre-stream-idle ACT), then bulk.  h0 vtb
            casts interleave on DVE."""
            proj_chunk(0, 0, 0, 128, "dve")
            proj_chunk(0, 0, 128, 128, "dve")
            proj_chunk(0, 1, 0, 256, "act")
            proj_chunk(0, 0, 256, 768, "dve")
            proj_chunk(0, 0, 1024, 1024, "dve")
            proj_chunk(0, 1, 256, 768, "dve")
            proj_chunk(0, 1, 1024, 1024, "dve")

        def emit_vproj(h, j0, nj):
            """v chunks j0..j0+nj-1 -> vaug natural layout."""
            w = nj * 128
            bigt = ps.tile([D, TILE], F32, tag="big", bufs=3,
                           name=f"vps_{h}_{j0}")
            for c in range(0, w, 128):
                nc.tensor.matmul(
                    bigt[:, c : c + 128],
                    vtb[h][:, j0 * 128 + c : j0 * 128 + c + 128],
                    wvb[h][:],
                    start=True,
                    stop=True,
                )
            nc.vector.tensor_copy(
                vaug[h][:, j0 : j0 + nj, 0:128],
                bigt[:, 0:w].rearrange("p (j c) -> p j c", c=128),
            )

        ctx_box = [None]

        def emit_pv(h, i, last_head):
            if i % 2 == 0:
                ctx_box[0] = ps.tile([D, 2, 132], F32, tag="ctx", bufs=2,
                                     name=f"ctx_{h}_{i}")
            ctxp = ctx_box[0]
            sl = i % 2
            for j in range(i + 1):
                p = OFFS[j] + 128 * (i - j)
                nc.tensor.matmul(
                    ctxp[:, sl, 0:129],
                    exp_all[h][:, p : p + 128],
                    vaug[h][:, j, 0:129],
                    start=(j == 0),
                    stop=(j == i),
                )
            recip = sb.tile([D, 1], F32, tag="recip", bufs=4,
                            name=f"recip_{h}_{i}")
            nc.vector.reciprocal(recip[:], ctxp[:, sl, 128:129])
            nc.vector.tensor_scalar_mul(
                osb[h][:, i, :], ctxp[:, sl, 0:128], recip[:]
            )
            # milestone out-DMAs; split the tail bands on the last head so
            # the final DMA is small
            def dma_bands(b0, b1):
                nc.sync.dma_start(
                    out_d[h, b0 * 128 : b1 * 128, :].rearrange(
                        "(b s) e -> s b e", b=b1 - b0
                    ),
                    osb[h][:, b0:b1, :],
                )
            if not last_head:
                if i in (3, 7, 11, 15):
                    dma_bands(i - 3, i + 1)
            else:
                # fire per band from 8 on so the final transfer is tiny;
                # late ones go out the otherwise-idle ACT queue
                if i in (3, 7):
                    dma_bands(i - 3, i + 1)
                elif 8 <= i <= 9:
                    dma_bands(i, i + 1)
                elif 10 <= i <= 14:
                    nc.scalar.dma_start(
                        out_d[h, i * 128 : (i + 1) * 128, :], osb[h][:, i, :]
                    )
                elif i == 15:
                    nc.scalar.dma_start(out_d[h, 15 * 128 : 15 * 128 + 128, 0:64],
                                        osb[h][:, 15, 0:64])
                    nc.scalar.dma_start(out_d[h, 15 * 128 : 15 * 128 + 128, 64:128],
                                        osb[h][:, 15, 64:128])

        def emit_score_tile(h, tiles, t):
            t0, w = tiles[t]
            bigt = ps.tile([D, TILE], F32, tag="big", bufs=3,
                           name=f"scps_{h}_{t}")
            pos = t0
            while pos < t0 + w:
                j = 0
                while OFFS[j + 1] <= pos:
                    j += 1
                bank_end = t0 + ((pos - t0) // 512 + 1) * 512
                end = min(OFFS[j + 1], t0 + w, bank_end)
                qcol = 128 * j + (pos - OFFS[j])
                is_start = pos == OFFS[j]
                nc.tensor.matmul(
                    bigt[:, pos - t0 : end - t0],
                    qkmt[h][:, S + 128 * j : S + 128 * (j + 1)],
                    qkmt[h][:, qcol : qcol + (end - pos)],
                    start=True,
                    stop=not is_start,
                )
                if is_start:
                    nc.tensor.matmul(
                        bigt[:, pos - t0 : pos - t0 + 128],
                        identb[:],
                        maskb[:],
                        start=False,
                        stop=True,
                    )
                pos = end
            nc.scalar.activation(
                exp_all[h][:, t0 : t0 + w], bigt[:, 0:w], AF.Exp,
                scale=1.0 / NF,
            )

        def emit_stream(h, tiles, band_tile, interleave, pv_start_t,
                        first_t=0):
            """scores/exp/PV stream for head h.  `interleave` maps tile
            index -> list of callables to emit after that tile's exp.
            Tiles < first_t were already emitted elsewhere."""
            issued = 0  # bands 0..issued-1 already emitted
            for t in range(first_t, len(tiles)):
                emit_score_tile(h, tiles, t)
                for fn in interleave.get(t, ()):
                    fn()
                if t >= pv_start_t:
                    while issued < NB and band_tile[issued] <= t - 1:
                        emit_pv(h, issued, h == HPC - 1)
                        issued += 1
            while issued < NB:
                emit_pv(h, issued, h == HPC - 1)
                issued += 1

        # ---- head 0: proj pre-stream, h1-proj interleaved late --------
        emit_qk_proj_h0()

        def h1_proj_piece(part, half):
            def fn():
                base = part * S
                bigt = ps.tile([D, TILE], F32, tag="big", bufs=3,
                               name=f"projps1_{part}_{half}")
                for c in range(2):
                    c0 = half * 1024 + c * 512
                    nc.tensor.matmul(
                        bigt[:, c * 512 : c * 512 + 512],
                        wqk_t[1][:],
                        qkraw[1][:, base + c0 : base + c0 + 512],
                        start=True,
                        stop=True,
                    )
                nc.vector.tensor_copy(
                    qkmt[1][:, base + half * 1024 : base + half * 1024 + 1024],
                    bigt[:, 0:1024],
                )
            return fn

        tiles0, band_tile0 = make_tiles(lead_small=True, tail_cuts=False)
        tiles1, band_tile1 = make_tiles(lead_small=False, tail_cuts=False)
        n0 = len(tiles0)
        inter0 = {}

        def add_inter(t, fn):
            inter0.setdefault(t, []).append(fn)

        add_inter(3, lambda: emit_vproj(0, 0, 8))
        add_inter(4, lambda: emit_vproj(0, 8, 8))
        add_inter(6, h1_proj_piece(0, 0))
        add_inter(7, h1_proj_piece(0, 1))
        add_inter(8, h1_proj_piece(1, 0))
        add_inter(9, h1_proj_piece(1, 1))
        add_inter(11, lambda: emit_vproj(1, 0, 8))
        add_inter(12, lambda: emit_vproj(1, 8, 8))
        # feed h1's first score tiles into h0's tail so ACT never idles
        # across the head transition
        for k in range(4):
            add_inter(n0 - 4 + k,
                      (lambda kk: lambda: emit_score_tile(1, tiles1, kk))(k))
        emit_stream(0, tiles0, band_tile0, inter0, pv_start_t=4)

        # ---- head 1 (proj, vproj, tiles 0-4 already emitted above) ----
        emit_score_tile(1, tiles1, 4)
        emit_stream(1, tiles1, band_tile1, {}, pv_start_t=5, first_t=5)

    nc.compile()
    return nc


_NC_CACHE = None


def _get_program():
    global _NC_CACHE
    if _NC_CACHE is None:
        _NC_CACHE = build_program()
    return _NC_CACHE


def make_in_maps(query_layer, key_layer, value_layer, svd_qk, svd_v):
    bf = ml_dtypes.bfloat16
    qt = np.ascontiguousarray(query_layer[:, 0].transpose(1, 2, 0)).astype(bf)
    kt = np.ascontiguousarray(key_layer[:, 0].transpose(1, 2, 0)).astype(bf)
    vt = np.ascontiguousarray(value_layer[:, 0].transpose(1, 2, 0)).astype(bf)
    svd_qk = np.ascontiguousarray(svd_qk).astype(bf)
    svd_v = np.ascontiguousarray(svd_v).astype(bf)

    identb = np.eye(D, dtype=ml_dtypes.bfloat16)
    r = np.arange(D)
    maskb = np.where(r[:, None] > r[None, :], NEG, 0.0).astype(ml_dtypes.bfloat16)

    in_maps = []
    for c in range(NCORES):
        hs = slice(c * HPC, (c + 1) * HPC)
        in_maps.append(
            {
                "qt": qt[hs],
                "kt": kt[hs],
                "vt": vt[hs],
                "wqk": svd_qk[hs],
                "wv": svd_v[hs],
                "identb": identb,
                "maskb": maskb,
            }
        )
    return in_maps


def assemble_output(results):
    out = np.empty((S, B, H * D), dtype=np.float32)
    for c in range(NCORES):
        o = np.asarray(results[c]["out"], dtype=np.float32)  # [HPC, S, D]
        for hl in range(HPC):
            h = c * HPC + hl
            out[:, 0, h * D : (h + 1) * D] = o[hl]
    return out


def kernel(query_layer, key_layer, value_layer, attention_mask, svd_qk, svd_v):
    nc = _get_program()
    in_maps = make_in_maps(query_layer, key_layer, value_layer, svd_qk, svd_v)
    res = run_bass_kernel_spmd(nc, in_maps, list(range(NCORES))).results
    return assemble_output(res)


# revision 47
# speedup vs baseline: 1.0432x; 1.0102x over previous
"""Trainium2 Bass kernel for nn_CoreAttention (S=2048, B=1, H=16, D=128).

Sharding: 16 heads across 8 NeuronCores (2 heads/core, tensor parallel).

Design: flash-style causal stream per head with a natural-output PV.

  qk_proj: psum = Wqk^T @ [Q^T | K^T]       (bf16 PE, host pre-casts all
                                             inputs to bf16)
  v:       v_nat[s,e] = V^T-chunk^T @ Wv    stored as vaug[k, j, 0:129]
                                             with column 128 = 1.0
  scoresT[k,q] = kmt_j^T @ qmt chunks       (causal column stream in
                                             1024-col psum tiles x3;
                                             diag mask via PE accumulate
                                             of a -1e4 tile)
  expT   = exp(scoresT / NF)                (ACT, scale folded in, bf16)
  PV band i: ctx[q, 0:129] = sum_j expT_block(i,j)^T @ vaug_j
             column 128 = the softmax sums (ones column of vaug)
  out    = ctx[:, 0:128] * (1/ctx[:, 128])  (DVE recip + per-partition
                                             scalar multiply, bf16 out,
                                             host upcasts)

Schedule highlights (all tuned against neuron-profile traces):
  - input DMAs split across the SP and ACT HWDGE queues, lead chunks
    first, so the stream starts ~8us in; PE p-state warmup matmuls fill
    the DMA-wait window
  - head-1's projections, v-projection and first score tiles are
    interleaved into head-0's stream so ACT (the pacing engine, ~39us
    of exp) never idles across the head transition
  - PV bands pair up in [128, 2, 132] psum tiles; per-head output
    staging so out-DMA issue never backpressures the epilogue
"""

import sys
from contextlib import ExitStack

import numpy as np

for _p in ("/opt/trn_rl_repo",):
    if _p not in sys.path:
        sys.path.insert(0, _p)

import ml_dtypes
import concourse.bass as bass
import concourse.tile as tile
from concourse import bacc, mybir
from concourse.bass_utils import run_bass_kernel_spmd

S, B, H, D = 2048, 1, 16, 128
HPC = 2  # heads per core
NCORES = 8
NB = S // 128  # 16 seq blocks of 128
NF = float(np.sqrt(2048.0 / 16.0))  # NORM_FACTOR
NEG = -10000.0
TILE = 1024  # psum scores tile columns (2 banks)

F32 = mybir.dt.float32
F32R = mybir.dt.float32r
BF16 = mybir.dt.bfloat16
AF = mybir.ActivationFunctionType

# causal column stream: pass j emits scoresT columns for q in [128j, S)
OFFS = [0]
for j in range(NB):
    OFFS.append(OFFS[-1] + (S - 128 * j))
W = OFFS[NB]  # 17408 total causal columns per head


def build_program() -> bass.Bass:
    nc = bacc.Bacc(
        "TRN2", target_bir_lowering=False, debug=False, num_devices=NCORES
    )

    qt_d = nc.dram_tensor("qt", [HPC, D, S], BF16, kind="ExternalInput")
    kt_d = nc.dram_tensor("kt", [HPC, D, S], BF16, kind="ExternalInput")
    vt_d = nc.dram_tensor("vt", [HPC, D, S], BF16, kind="ExternalInput")
    wqk_d = nc.dram_tensor("wqk", [HPC, D, D], BF16, kind="ExternalInput")
    wv_d = nc.dram_tensor("wv", [HPC, D, D], BF16, kind="ExternalInput")
    identb_d = nc.dram_tensor("identb", [D, D], BF16, kind="ExternalInput")
    maskb_d = nc.dram_tensor("maskb", [D, D], BF16, kind="ExternalInput")
    out_d = nc.dram_tensor("out", [HPC, S, D], BF16, kind="ExternalOutput")

    def make_tiles(lead_small, tail_cuts):
        """Column tiling of the causal stream.  Small lead tiles let the
        stream start while the bulk of q is still in flight; tail cuts
        at the last bands' diagonals release PV(13..15) progressively."""
        widths = ([512, 512, 512] if lead_small else [])
        pos = sum(widths)
        while pos < W:
            w = min(TILE, W - pos)
            widths.append(w)
            pos += w
        tiles = []
        pos = 0
        for w in widths:
            if tail_cuts:
                for cut in (OFFS[13], OFFS[14], OFFS[15]):
                    if pos < cut < pos + w:
                        tiles.append((pos, cut - pos))
                        w -= cut - pos
                        pos = cut
            tiles.append((pos, w))
            pos += w
        band_tile = []
        for i in range(NB):
            for t, (t0, w) in enumerate(tiles):
                if t0 <= OFFS[i] < t0 + w:
                    band_tile.append(t)
                    break
        return tiles, band_tile

    with tile.TileContext(nc) as tc, ExitStack() as ctx:
        cpool = ctx.enter_context(tc.tile_pool(name="const", bufs=1))
        sb = ctx.enter_context(tc.tile_pool(name="sb", bufs=1))
        ps = ctx.enter_context(tc.tile_pool(name="ps", bufs=1, space="PSUM"))

        # ---- input DMAs: h0's critical path leads both HWDGE queues ---
        # SP: wqk0 + q0 lead/bulk, v0, then all of h1.  ACT: k0 + consts
        # (ACT queue starts in parallel with SP's, and ACT is idle until
        # the first exp).
        wqk_t, qkraw, vtb, wvb = {}, {}, {}, {}
        for h in range(HPC):
            wqk_t[h] = sb.tile([D, D], BF16, tag="wqk", bufs=2, name=f"wqk_{h}")
            wvb[h] = sb.tile([D, D], BF16, tag="wvb", bufs=2, name=f"wvb_{h}")
            qkraw[h] = sb.tile([D, 2 * S], BF16, tag="qkraw", bufs=2,
                               name=f"qkraw_{h}")
            vtb[h] = sb.tile([D, S], BF16, tag="vtb", bufs=2, name=f"vtb_{h}")
        identb = cpool.tile([D, D], BF16)
        nc.scalar.dma_start(identb[:], identb_d[:])
        maskb = cpool.tile([D, D], BF16)
        nc.scalar.dma_start(maskb[:], maskb_d[:])
        nc.scalar.dma_start(qkraw[0][:, S : S + 256], kt_d[0][:, 0:256])
        nc.scalar.dma_start(qkraw[0][:, S + 256 : S + 1024],
                            kt_d[0][:, 256:1024])
        nc.scalar.dma_start(qkraw[0][:, S + 1024 : 2 * S],
                            kt_d[0][:, 1024:2048])
        nc.sync.dma_start(wqk_t[0][:], wqk_d[0])
        nc.sync.dma_start(qkraw[0][:, 0:256], qt_d[0][:, 0:256])
        nc.sync.dma_start(qkraw[0][:, 256:1024], qt_d[0][:, 256:1024])
        nc.sync.dma_start(qkraw[0][:, 1024:2048], qt_d[0][:, 1024:2048])
        nc.sync.dma_start(wvb[0][:], wv_d[0])
        nc.sync.dma_start(vtb[0][:], vt_d[0])
        nc.sync.dma_start(wqk_t[1][:], wqk_d[1])
        nc.sync.dma_start(wvb[1][:], wv_d[1])
        nc.sync.dma_start(qkraw[1][:, 0:2048], qt_d[1])
        nc.sync.dma_start(qkraw[1][:, S : 2 * S], kt_d[1])
        nc.sync.dma_start(vtb[1][:], vt_d[1])

        # ---- working tiles; gpsimd only seeds the vaug ones column ----
        vaug, qkmt, exp_all, osb = {}, {}, {}, {}
        for h in range(HPC):
            vaug[h] = sb.tile([D, NB, 132], BF16, tag="vaug", bufs=2,
                              name=f"vaug_{h}")
            qkmt[h] = sb.tile([D, 2 * S], BF16, tag="qkmt", bufs=2,
                              name=f"qkmt_{h}")
            exp_all[h] = sb.tile([D, W], BF16, tag="expall", bufs=2,
                                 name=f"expall_{h}")
            osb[h] = sb.tile([D, NB, 128], BF16, tag="osb", bufs=2,
                             name=f"osb_{h}")
            nc.gpsimd.memset(vaug[h][:, :, 128:129], 1.0)

        # ---- PE p-state warmup: garbage matmuls fill the DMA-wait ----
        # window so the PE is at full clock when real data lands
        junk = sb.tile([D, 512], BF16, tag="junk", bufs=1, name="junk")
        nc.gpsimd.memset(junk[:], 1.0)
        for wi in range(8):
            warm_ps = ps.tile([D, TILE], F32, tag="big", bufs=3,
                              name=f"warm_{wi}")
            nc.tensor.matmul(warm_ps[:, 0:512], junk[:, 0:128], junk[:],
                             start=True, stop=True)

        def proj_chunk(h, part, c0, w, copy_eng):
            """one bf16 projection chunk psum tile + drain copy."""
            base = part * S
            bigt = ps.tile([D, TILE], F32, tag="big", bufs=3,
                           name=f"projps_{h}_{part}_{c0}")
            for c in range(0, w, 512):
                cw = min(512, w - c)
                nc.tensor.matmul(
                    bigt[:, c : c + cw],
                    wqk_t[h][:],
                    qkraw[h][:, base + c0 + c : base + c0 + c + cw],
                    start=True,
                    stop=True,
                )
            if copy_eng == "act":
                nc.scalar.activation(
                    qkmt[h][:, base + c0 : base + c0 + w], bigt[:, 0:w],
                    AF.Copy,
                )
            else:
                nc.vector.tensor_copy(
                    qkmt[h][:, base + c0 : base + c0 + w], bigt[:, 0:w]
                )

        def emit_qk_proj_h0():
            """head-0 projections matched to DMA chunk arrival: q/k leads
            (k-lead copy on pre-stream-idle ACT), then bulk.  h0 vtb
            casts interleave on DVE."""
            proj_chunk(0, 0, 0, 256, "dve")
            proj_chunk(0, 1, 0, 256, "act")
            proj_chunk(0, 0, 256, 768, "dve")
            proj_chunk(0, 0, 1024, 1024, "dve")
            proj_chunk(0, 1, 256, 768, "dve")
            proj_chunk(0, 1, 1024, 1024, "dve")

        def emit_vproj(h, j0, nj):
            """v chunks j0..j0+nj-1 -> vaug natural layout."""
            w = nj * 128
            bigt = ps.tile([D, TILE], F32, tag="big", bufs=3,
                           name=f"vps_{h}_{j0}")
            for c in range(0, w, 128):
                nc.tensor.matmul(
                    bigt[:, c : c + 128],
                    vtb[h][:, j0 * 128 + c : j0 * 128 + c + 128],
                    wvb[h][:],
                    start=True,
                    stop=True,
                )
            nc.vector.tensor_copy(
                vaug[h][:, j0 : j0 + nj, 0:128],
                bigt[:, 0:w].rearrange("p (j c) -> p j c", c=128),
            )

        ctx_box = [None]

        def emit_pv(h, i, last_head):
            if i % 2 == 0:
                ctx_box[0] = ps.tile([D, 2, 132], F32, tag="ctx", bufs=2,
                                     name=f"ctx_{h}_{i}")
            ctxp = ctx_box[0]
            sl = i % 2
            for j in range(i + 1):
                p = OFFS[j] + 128 * (i - j)
                nc.tensor.matmul(
                    ctxp[:, sl, 0:129],
                    exp_all[h][:, p : p + 128],
                    vaug[h][:, j, 0:129],
                    start=(j == 0),
                    stop=(j == i),
                )
            recip = sb.tile([D, 1], F32, tag="recip", bufs=4,
                            name=f"recip_{h}_{i}")
            nc.vector.reciprocal(recip[:], ctxp[:, sl, 128:129])
            nc.vector.tensor_scalar_mul(
                osb[h][:, i, :], ctxp[:, sl, 0:128], recip[:]
            )
            # milestone out-DMAs; split the tail bands on the last head so
            # the final DMA is small
            def dma_bands(b0, b1):
                nc.sync.dma_start(
                    out_d[h, b0 * 128 : b1 * 128, :].rearrange(
                        "(b s) e -> s b e", b=b1 - b0
                    ),
                    osb[h][:, b0:b1, :],
                )
            if not last_head:
                if i in (3, 7, 11, 15):
                    dma_bands(i - 3, i + 1)
            else:
                # fire per band from 8 on so the final transfer is tiny;
                # late ones go out the otherwise-idle ACT queue
                if i in (3, 7):
                    dma_bands(i - 3, i + 1)
                elif 8 <= i <= 9:
                    dma_bands(i, i + 1)
                elif 10 <= i <= 14:
                    nc.scalar.dma_start(
                        out_d[h, i * 128 : (i + 1) * 128, :], osb[h][:, i, :]
                    )
                elif i == 15:
                    nc.scalar.dma_start(out_d[h, 15 * 128 : 15 * 128 + 128, 0:64],
                                        osb[h][:, 15, 0:64])
                    nc.scalar.dma_start(out_d[h, 15 * 128 : 15 * 128 + 128, 64:128],
                                        osb[h][:, 15, 64:128])

        def emit_score_tile(h, tiles, t):
            t0, w = tiles[t]
            bigt = ps.tile([D, TILE], F32, tag="big", bufs=3,
                           name=f"scps_{h}_{t}")
            pos = t0
            while pos < t0 + w:
                j = 0
                while OFFS[j + 1] <= pos:
                    j += 1
                bank_end = t0 + ((pos - t0) // 512 + 1) * 512
                end = min(OFFS[j + 1], t0 + w, bank_end)
                qcol = 128 * j + (pos - OFFS[j])
                is_start = pos == OFFS[j]
                nc.tensor.matmul(
                    bigt[:, pos - t0 : end - t0],
                    qkmt[h][:, S + 128 * j : S + 128 * (j + 1)],
                    qkmt[h][:, qcol : qcol + (end - pos)],
                    start=True,
                    stop=not is_start,
                )
                if is_start:
                    nc.tensor.matmul(
                        bigt[:, pos - t0 : pos - t0 + 128],
                        identb[:],
                        maskb[:],
                        start=False,
                        stop=True,
                    )
                pos = end
            nc.scalar.activation(
                exp_all[h][:, t0 : t0 + w], bigt[:, 0:w], AF.Exp,
                scale=1.0 / NF,
            )

        def emit_stream(h, tiles, band_tile, interleave, pv_start_t,
                        first_t=0):
            """scores/exp/PV stream for head h.  `interleave` maps tile
            index -> list of callables to emit after that tile's exp.
            Tiles < first_t were already emitted elsewhere."""
            issued = 0  # bands 0..issued-1 already emitted
            for t in range(first_t, len(tiles)):
                emit_score_tile(h, tiles, t)
                for fn in interleave.get(t, ()):
                    fn()
                if t >= pv_start_t:
                    while issued < NB and band_tile[issued] <= t - 1:
                        emit_pv(h, issued, h == HPC - 1)
                        issued += 1
            while issued < NB:
                emit_pv(h, issued, h == HPC - 1)
                issued += 1

        # ---- head 0: proj pre-stream, h1-proj interleaved late --------
        emit_qk_proj_h0()

        def h1_proj_piece(part, half):
            def fn():
                base = part * S
                bigt = ps.tile([D, TILE], F32, tag="big", bufs=3,
                               name=f"projps1_{part}_{half}")
                for c in range(2):
                    c0 = half * 1024 + c * 512
                    nc.tensor.matmul(
                        bigt[:, c * 512 : c * 512 + 512],
                        wqk_t[1][:],
                        qkraw[1][:, base + c0 : base + c0 + 512],
                        start=True,
                        stop=True,
                    )
                nc.vector.tensor_copy(
                    qkmt[1][:, base + half * 1024 : base + half * 1024 + 1024],
                    bigt[:, 0:1024],
                )
            return fn

        tiles0, band_tile0 = make_tiles(lead_small=True, tail_cuts=False)
        tiles1, band_tile1 = make_tiles(lead_small=False, tail_cuts=False)
        n0 = len(tiles0)
        inter0 = {}

        def add_inter(t, fn):
            inter0.setdefault(t, []).append(fn)

        add_inter(3, lambda: emit_vproj(0, 0, 8))
        add_inter(4, lambda: emit_vproj(0, 8, 8))
        add_inter(6, h1_proj_piece(0, 0))
        add_inter(7, h1_proj_piece(0, 1))
        add_inter(8, h1_proj_piece(1, 0))
        add_inter(9, h1_proj_piece(1, 1))
        add_inter(11, lambda: emit_vproj(1, 0, 8))
        add_inter(12, lambda: emit_vproj(1, 8, 8))
        # feed h1's first score tiles into h0's tail so ACT never idles
        # across the head transition
        for k in range(4):
            add_inter(n0 - 4 + k,
                      (lambda kk: lambda: emit_score_tile(1, tiles1, kk))(k))
        emit_stream(0, tiles0, band_tile0, inter0, pv_start_t=4)

        # ---- head 1 (proj, vproj, tiles 0-4 already emitted above) ----
        emit_score_tile(1, tiles1, 4)
        emit_stream(1, tiles1, band_tile1, {}, pv_start_t=5, first_t=5)

    nc.compile()
    return nc


_NC_CACHE = None


def _get_program():
    global _NC_CACHE
    if _NC_CACHE is None:
        _NC_CACHE = build_program()
    return _NC_CACHE


def make_in_maps(query_layer, key_layer, value_layer, svd_qk, svd_v):
    bf = ml_dtypes.bfloat16
    qt = np.ascontiguousarray(query_layer[:, 0].transpose(1, 2, 0)).astype(bf)
    kt = np.ascontiguousarray(key_layer[:, 0].transpose(1, 2, 0)).astype(bf)
    vt = np.ascontiguousarray(value_layer[:, 0].transpose(1, 2, 0)).astype(bf)
    svd_qk = np.ascontiguousarray(svd_qk).astype(bf)
    svd_v = np.ascontiguousarray(svd_v).astype(bf)

    identb = np.eye(D, dtype=ml_dtypes.bfloat16)
    r = np.arange(D)
    maskb = np.where(r[:, None] > r[None, :], NEG, 0.0).astype(ml_dtypes.bfloat16)

    in_maps = []
    for c in range(NCORES):
        hs = slice(c * HPC, (c + 1) * HPC)
        in_maps.append(
            {
                "qt": qt[hs],
                "kt": kt[hs],
                "vt": vt[hs],
                "wqk": svd_qk[hs],
                "wv": svd_v[hs],
                "identb": identb,
                "maskb": maskb,
            }
        )
    return in_maps


def assemble_output(results):
    out = np.empty((S, B, H * D), dtype=np.float32)
    for c in range(NCORES):
        o = np.asarray(results[c]["out"], dtype=np.float32)  # [HPC, S, D]
        for hl in range(HPC):
            h = c * HPC + hl
            out[:, 0, h * D : (h + 1) * D] = o[hl]
    return out


def kernel(query_layer, key_layer, value_layer, attention_mask, svd_qk, svd_v):
    nc = _get_program()
    in_maps = make_in_maps(query_layer, key_layer, value_layer, svd_qk, svd_v)
    res = run_bass_kernel_spmd(nc, in_maps, list(range(NCORES))).results
    return assemble_output(res)
